# revision 1
# baseline (speedup 1.0000x reference)
"""Trainium2 Bass kernel for AutoregressiveConvLSTM log-prob.

Strategy
--------
Data-parallel over batch: 64 images -> 8 NeuronCores, 8 images each.

Per-core layout: each (image-batch, channel) "plane" is an SBUF tile
[H=128 partitions, 1042 free] where image b occupies flat columns
130*b+1 .. 130*b+128 and the surrounding columns are zero pads.

All 3x3 convs run on the TensorEngine as banded matmuls:
  out[h_out, col] = sum_h_in Band[h_in, h_out] * plane[h_in, col+dx]
where Band is a [128,128] tri-diagonal matrix holding the three dy taps
(built on the host from the conv weights) and the dx in {-1,0,1} shift
is a free-dim AP offset into the zero pads.  Contributions over
(cin, dx) accumulate in PSUM.  Matmuls use float32r (full fp32 data,
fast PE mode).  LSTM pointwise math runs on ScalarE/VectorE in fp32.

The per-pixel log-prob terms are reduced over W on VectorE into a
[128 (=H), 8 (=image)] accumulator, and over H at the end with a single
ones-vector matmul.  Output per core: [8] -> host concatenates to [64].
"""

import numpy as np

B_FULL, C, H, W, F = 64, 16, 128, 128, 2
NCORES = 8
BL = B_FULL // NCORES            # images per core
WB = W + 2                       # per-image block width incl. pads
FREE = BL * WB + 2               # flat free size (+2 spare zero cols)
HALF_LOG_2PI = 0.9189385332046727

# chunks: (b0, n_imgs, c0, ilo)  — psum columns [c0, c0+n*WB), image b
# starts at local column WB*(b-b0)+ilo, interior slice [ilo, ilo+128)
CHUNKS = [(0, 3, 1, 0), (3, 3, 3 * WB, 1), (6, 2, 6 * WB, 1)]

N_STEP_BANDS = 3 + 72 + 12 + 12          # u, gates, head1, head2
N_ONETIME_BANDS = 96 + 12 + 12           # cond1, cond2, partial1
NB = N_ONETIME_BANDS + N_STEP_BANDS


def _band(w3):
    """[128,128] B[h_in,h_out] = w3[h_in-h_out+1] (tri-diagonal)."""
    b = np.zeros((H, H), np.float32)
    for dy in (-1, 0, 1):
        ar = np.arange(max(0, -dy), H - max(0, dy))
        b[ar + dy, ar] = w3[dy + 1]
    return b


def _build_bands(Wci, Wc1, Wc2, Wo1, Wo2, Wih, Whh):
    bands = np.zeros((NB, H, H), np.float32)
    i = 0
    # one-time: cond1 (16->2, ci-major for group streaming), cond2,
    # partial1 (cond_f part of Wo1)
    for ci in range(16):
        for co in range(2):
            for dx in range(3):
                bands[i] = _band(Wc1[:, dx, ci, co]); i += 1
    for co in range(2):
        for ci in range(2):
            for dx in range(3):
                bands[i] = _band(Wc2[:, dx, ci, co]); i += 1
    for co in range(2):
        for ci in range(2):
            for dx in range(3):
                bands[i] = _band(Wo1[:, dx, 2 + ci, co]); i += 1
    assert i == N_ONETIME_BANDS
    # step bands: u conv (1->1)
    for dx in range(3):
        bands[i] = _band(Wci[:, dx, 0, 0]); i += 1
    # gates: src 0,1 = h planes (Whh), src 2 = u plane (Wih)
    for co in range(8):
        for src in range(3):
            for dx in range(3):
                w3 = Whh[:, dx, src, co] if src < 2 else Wih[:, dx, 0, co]
                bands[i] = _band(w3); i += 1
    # head1 (h part of Wo1), head2 (Wo2)
    for co in range(2):
        for ci in range(2):
            for dx in range(3):
                bands[i] = _band(Wo1[:, dx, ci, co]); i += 1
    for co in range(2):
        for ci in range(2):
            for dx in range(3):
                bands[i] = _band(Wo2[:, dx, ci, co]); i += 1
    assert i == NB
    return bands


def _build_program(bci, bc1, bc2, bo1, bo2, bih):
    import concourse.bacc as bacc
    import concourse.mybir as mybir
    import concourse.tile as tile

    f32 = mybir.dt.float32
    MM = mybir.dt.float32r
    AF = mybir.ActivationFunctionType
    OP = mybir.AluOpType
    AX = mybir.AxisListType

    nc = bacc.Bacc("TRN2", target_bir_lowering=False, debug=False)
    xd = nc.dram_tensor("x", [BL, C, H, W], MM, kind="ExternalInput")
    cd = nc.dram_tensor("cond", [BL, C, H, W], MM, kind="ExternalInput")
    bd = nc.dram_tensor("bands", [NB, H, H], MM, kind="ExternalInput")
    od = nc.dram_tensor("out", [BL, 1], f32, kind="ExternalOutput")

    def i3(ap_flat, b0, n, lo):
        # [128, n, 128] interior view of a [128, >=1040] flat AP
        return ap_flat[:, : BL * WB].rearrange(
            "p (b w) -> p b w", w=WB)[:, b0:b0 + n, lo:lo + 128]

    with tile.TileContext(nc) as tc:
        import contextlib
        ctx = contextlib.ExitStack()
        with ctx:
            state = ctx.enter_context(tc.tile_pool(name="state", bufs=1))
            sbands = ctx.enter_context(tc.tile_pool(name="sbands", bufs=1))
            stream = ctx.enter_context(tc.tile_pool(name="stream", bufs=3))
            ctmp = ctx.enter_context(tc.tile_pool(name="ctmp", bufs=2))
            tmp = ctx.enter_context(tc.tile_pool(name="tmp", bufs=16))
            psum = ctx.enter_context(
                tc.tile_pool(name="psum", bufs=8, space="PSUM"))

            # step bands, resident
            sb = sbands.tile([H, N_STEP_BANDS * H], MM, tag="sb", name="sb")
            for k in range(8):
                s = (N_STEP_BANDS * k) // 8
                e = (N_STEP_BANDS * (k + 1)) // 8
                nc.sync.dma_start(
                    sb[:, s * H:e * H],
                    bd[N_ONETIME_BANDS + s:N_ONETIME_BANDS + e].rearrange(
                        "n p m -> p n m"))

            def band_st(i):
                return sb[:, i * H:(i + 1) * H].bitcast(MM)

            # persistent planes
            def plane(tag, dt=MM, memset=True):
                t = state.tile([H, FREE], dt, tag=tag)
                if memset:
                    nc.vector.memset(t[:].bitcast(f32), 0.0)
                return t

            h_pl = [plane("h0"), plane("h1")]
            c_pl = [plane("c0", f32), plane("c1", f32)]
            u_pl = plane("u")
            r_pl = [plane("r0"), plane("r1")]
            p1_pl = [plane("p1a", f32), plane("p1b", f32)]
            lp = state.tile([H, BL], f32, tag="lp", name="lp")
            nc.vector.memset(lp[:], 0.0)
            ones = state.tile([H, 1], f32, tag="ones", name="ones")
            nc.vector.memset(ones[:], 1.0)
            # bias columns: 0-7 bih, 8-9 bc1, 10-11 bc2, 12-13 bo1, 14 bci,
            # 15 = -bo2[1], 16 = final output bias
            cst = -16.0 * 128.0 * 128.0 * (float(bo2[1]) + HALF_LOG_2PI)
            bias_vals = (list(bih) + list(bc1) + list(bc2) + list(bo1)
                         + [float(bci[0]), -float(bo2[1]), cst])
            bias_t = state.tile([H, 17], f32, tag="bias", name="bias")
            for j, v in enumerate(bias_vals):
                nc.vector.memset(bias_t[:, j:j + 1], float(v))

            def bap(j, p=H):
                return bias_t[:p, j:j + 1]

            def new_plane(pool, src_dram, ci, tag, bufs=None):
                t = pool.tile([H, FREE], MM, tag=tag, name=tag, bufs=bufs)
                t3 = t[:, : BL * WB].rearrange("p (b w) -> p b w", w=WB)
                nc.vector.memset(t3[:, :, 0:1].bitcast(f32), 0.0)
                nc.vector.memset(t3[:, :, WB - 1:WB].bitcast(f32), 0.0)
                nc.vector.memset(t[:, BL * WB:].bitcast(f32), 0.0)
                nc.sync.dma_start(
                    t3[:, :, 1:129], src_dram[:, ci].rearrange("b h w -> h b w"))
                return t

            x_planes = {}

            def get_x(ci):
                if ci not in x_planes:
                    x_planes[ci] = new_plane(stream, xd, ci, "xpl")
                return x_planes[ci]

            # ---------------- cond phase ----------------
            with tc.tile_pool(name="otbands", bufs=2) as otp:
                GRP = 24
                ob_cur = [None]

                def load_group(g):
                    ob = otp.tile([H, GRP * H], MM, tag="ob", name="ob")
                    nc.sync.dma_start(
                        ob[:, :], bd[g * GRP:(g + 1) * GRP].rearrange(
                            "n p m -> p n m"))
                    ob_cur[0] = ob

                def band_ot(i):
                    j = i % GRP
                    return ob_cur[0][:, j * H:(j + 1) * H].bitcast(MM)

                # cond1: 16 -> 2, tanh
                pc = {}
                for co in range(2):
                    for k, (b0, n, c0, lo) in enumerate(CHUNKS):
                        pc[(co, k)] = psum.tile([H, 3 * WB], f32, tag="ps", name="ps")
                for ci in range(16):
                    if ci % 4 == 0:
                        load_group(ci // 4)
                    cpl = new_plane(stream, cd, ci, "cpl", bufs=2)
                    cf = cpl[:].bitcast(MM)
                    for co in range(2):
                        for k, (b0, n, c0, lo) in enumerate(CHUNKS):
                            for dx in (-1, 0, 1):
                                nc.tensor.matmul(
                                    pc[(co, k)][:, :n * WB],
                                    band_ot(ci * 6 + co * 3 + (dx + 1)),
                                    cf[:, c0 + dx:c0 + dx + n * WB],
                                    start=(ci == 0 and dx == -1),
                                    stop=(ci == 15 and dx == 1))
                tc_pl = [ctmp.tile([H, FREE], MM, tag="tc", name="tc") for _ in range(2)]
                for t in tc_pl:
                    nc.vector.memset(t[:].bitcast(f32), 0.0)
                for co in range(2):
                    for k, (b0, n, c0, lo) in enumerate(CHUNKS):
                        p3 = pc[(co, k)][:, :n * WB].rearrange(
                            "p (b w) -> p b w", w=WB)[:, :, lo:lo + 128]
                        nc.scalar.activation(
                            i3(tc_pl[co][:], b0, n, 1), p3, AF.Tanh,
                            bias=bap(8 + co))

                # cond2 -> cond_f planes; then partial1 = conv(cond_f)+bo1
                cf_pl = [ctmp.tile([H, FREE], MM, tag="cf", name="cf") for _ in range(2)]
                for t in cf_pl:
                    nc.vector.memset(t[:].bitcast(f32), 0.0)
                load_group(4)
                for dst, srcs, base, bias_col, out_pl in (
                        (cf_pl, tc_pl, 96, 10, None),
                        (None, cf_pl, 108, 12, p1_pl)):
                    tgt = dst if dst is not None else out_pl
                    for co in range(2):
                        for k, (b0, n, c0, lo) in enumerate(CHUNKS):
                            pq = psum.tile([H, 3 * WB], f32, tag="ps", name="ps")
                            first = True
                            for ci in range(2):
                                sf = srcs[ci][:].bitcast(MM)
                                for dx in (-1, 0, 1):
                                    nc.tensor.matmul(
                                        pq[:, :n * WB],
                                        band_ot(base + co * 6 + ci * 3 + dx + 1),
                                        sf[:, c0 + dx:c0 + dx + n * WB],
                                        start=first,
                                        stop=(ci == 1 and dx == 1))
                                    first = False
                            p3 = pq[:, :n * WB].rearrange(
                                "p (b w) -> p b w", w=WB)[:, :, lo:lo + 128]
                            nc.scalar.activation(
                                i3(tgt[co][:], b0, n, 1), p3, AF.Identity,
                                bias=bap(bias_col + co))

            # ---------------- steps ----------------
            def lp_tail(pq0, pq1, xt, b0, n, c0, lo):
                NN = n * WB
                e = tmp.tile([H, NN], f32, tag="tw", name="e")
                nc.scalar.activation(e[:], pq1[:, :NN], AF.Exp,
                                     bias=bap(15), scale=-1.0)
                d = tmp.tile([H, NN], f32, tag="tw", name="d")
                nc.vector.tensor_scalar(d[:], pq0[:, :NN], float(bo2[0]), None,
                                        OP.add)
                d2 = tmp.tile([H, NN], f32, tag="tw", name="d2")
                nc.vector.tensor_tensor(d2[:], xt[:, c0:c0 + NN].bitcast(f32), d[:],
                                        OP.subtract)
                z = tmp.tile([H, NN], f32, tag="tw", name="z")
                nc.vector.tensor_tensor(z[:], d2[:], e[:], OP.mult)
                s = tmp.tile([H, NN], f32, tag="tw", name="s")
                nc.scalar.activation(s[:], z[:], AF.Square,
                                     scale=0.7071067811865476)
                t = tmp.tile([H, NN], f32, tag="tw", name="t")
                nc.vector.tensor_tensor(t[:], s[:], pq1[:, :NN], OP.add)
                red = tmp.tile([H, n], f32, tag="tw", name="red")
                t3 = t[:].rearrange("p (b w) -> p b w", w=WB)[:, :, lo:lo + 128]
                nc.vector.reduce_sum(red[:], t3, AX.X)
                nc.vector.tensor_add(lp[:, b0:b0 + n], lp[:, b0:b0 + n], red[:])

            def head2_and_lp(xt_pl, b0, n, c0, lo):
                NN = n * WB
                pq = []
                for co in range(2):
                    q = psum.tile([H, 3 * WB], f32, tag="ps", name="ps")
                    first = True
                    for ci in range(2):
                        rf = r_pl[ci][:].bitcast(MM)
                        for dx in (-1, 0, 1):
                            nc.tensor.matmul(
                                q[:, :NN],
                                band_st(87 + co * 6 + ci * 3 + dx + 1),
                                rf[:, c0 + dx:c0 + dx + NN],
                                start=first, stop=(ci == 1 and dx == 1))
                            first = False
                    pq.append(q)
                lp_tail(pq[0], pq[1], xt_pl[:], b0, n, c0, lo)

            # step 0: feat = 0 -> r = relu(partial1)
            x0 = get_x(0)
            for (b0, n, c0, lo) in CHUNKS:
                for co in range(2):
                    nc.scalar.activation(
                        i3(r_pl[co][:], b0, n, 1),
                        i3(p1_pl[co][:], b0, n, 1), AF.Relu)
                head2_and_lp(x0, b0, n, c0, lo)

            for st in range(1, 16):
                xp = get_x(st - 1)
                xt = get_x(st)
                for (b0, n, c0, lo) in CHUNKS:
                    NN = n * WB
                    # u = conv(xp, Wci) + bci
                    pu = psum.tile([H, 3 * WB], f32, tag="ps", name="ps")
                    xf = xp[:].bitcast(MM)
                    for dx in (-1, 0, 1):
                        nc.tensor.matmul(pu[:, :NN], band_st(dx + 1),
                                         xf[:, c0 + dx:c0 + dx + NN],
                                         start=(dx == -1), stop=(dx == 1))
                    p3 = pu[:, :NN].rearrange(
                        "p (b w) -> p b w", w=WB)[:, :, lo:lo + 128]
                    nc.scalar.activation(i3(u_pl[:], b0, n, 1), p3,
                                         AF.Identity, bias=bap(14))
                    # gates
                    srcs = [h_pl[0], h_pl[1], u_pl]
                    pg = [None] * 8
                    for co in (0, 2, 4, 6, 1, 3, 5, 7):
                        g = psum.tile([H, 3 * WB], f32, tag="ps", name="ps")
                        first = True
                        for si, spl in enumerate(srcs):
                            sf = spl[:].bitcast(MM)
                            for dx in (-1, 0, 1):
                                nc.tensor.matmul(
                                    g[:, :NN],
                                    band_st(3 + co * 9 + si * 3 + dx + 1),
                                    sf[:, c0 + dx:c0 + dx + NN],
                                    start=first, stop=(si == 2 and dx == 1))
                                first = False
                        pg[co] = g
                    # LSTM pointwise (i,f,g,o = pg[0:2],[2:4],[4:6],[6:8])
                    for f in range(2):
                        ti = tmp.tile([H, NN], f32, tag="tw", name="ti")
                        nc.scalar.activation(ti[:], pg[f][:, :NN], AF.Sigmoid,
                                             bias=bap(f))
                        tg = tmp.tile([H, NN], f32, tag="tw", name="tg")
                        nc.scalar.activation(tg[:], pg[4 + f][:, :NN], AF.Tanh,
                                             bias=bap(4 + f))
                        tf = tmp.tile([H, NN], f32, tag="tw", name="tf")
                        nc.scalar.activation(tf[:], pg[2 + f][:, :NN],
                                             AF.Sigmoid, bias=bap(2 + f))
                        to = tmp.tile([H, NN], f32, tag="tw", name="to")
                        nc.scalar.activation(to[:], pg[6 + f][:, :NN],
                                             AF.Sigmoid, bias=bap(6 + f))
                        tig = tmp.tile([H, NN], f32, tag="tw", name="tig")
                        nc.vector.tensor_tensor(tig[:], ti[:], tg[:], OP.mult)
                        csl = c_pl[f][:, c0:c0 + NN]
                        nc.vector.tensor_tensor(csl, tf[:], csl, OP.mult)
                        nc.vector.tensor_tensor(csl, csl, tig[:], OP.add)
                        tc_ = tmp.tile([H, NN], f32, tag="tw", name="tc_")
                        nc.scalar.activation(tc_[:], csl, AF.Tanh)
                        to3 = to[:].rearrange(
                            "p (b w) -> p b w", w=WB)[:, :, lo:lo + 128]
                        tc3 = tc_[:].rearrange(
                            "p (b w) -> p b w", w=WB)[:, :, lo:lo + 128]
                        nc.vector.tensor_tensor(
                            i3(h_pl[f][:], b0, n, 1), to3, tc3, OP.mult)
                    # head1: r = relu(conv(h,Wo1[:, :, :2]) + partial1)
                    for co in range(2):
                        ph = psum.tile([H, 3 * WB], f32, tag="ps", name="ps")
                        first = True
                        for ci in range(2):
                            hf = h_pl[ci][:].bitcast(MM)
                            for dx in (-1, 0, 1):
                                nc.tensor.matmul(
                                    ph[:, :NN],
                                    band_st(75 + co * 6 + ci * 3 + dx + 1),
                                    hf[:, c0 + dx:c0 + dx + NN],
                                    start=first, stop=(ci == 1 and dx == 1))
                                first = False
                        hp = tmp.tile([H, NN], f32, tag="tw", name="hp")
                        nc.vector.tensor_tensor(
                            hp[:], ph[:, :NN], p1_pl[co][:, c0:c0 + NN], OP.add)
                        hp3 = hp[:].rearrange(
                            "p (b w) -> p b w", w=WB)[:, :, lo:lo + 128]
                        nc.scalar.activation(
                            i3(r_pl[co][:], b0, n, 1), hp3, AF.Relu)
                    head2_and_lp(xt, b0, n, c0, lo)

            # final: out = -(sum_p lp) - 16*128*128*(bo2[1] + HALF_LOG_2PI)
            po = psum.tile([BL, 1], f32, tag="ps", name="ps")
            nc.tensor.matmul(po[:], lp[:], ones[:], start=True, stop=True)
            osb = state.tile([BL, 1], f32, tag="osb", name="osb")
            nc.scalar.activation(osb[:], po[:], AF.Identity,
                                 scale=-1.0, bias=bap(16, BL))
            nc.sync.dma_start(od[:], osb[:])
    nc.compile()
    return nc


def kernel(**inputs):
    x = np.ascontiguousarray(inputs["x"], np.float32)
    cond = np.ascontiguousarray(inputs["cond"], np.float32)
    bands = _build_bands(
        np.asarray(inputs["Wci"], np.float32),
        np.asarray(inputs["Wc1"], np.float32),
        np.asarray(inputs["Wc2"], np.float32),
        np.asarray(inputs["Wo1"], np.float32),
        np.asarray(inputs["Wo2"], np.float32),
        np.asarray(inputs["Wih"], np.float32),
        np.asarray(inputs["Whh"], np.float32))
    nc = _build_program(
        np.asarray(inputs["bci"], np.float32),
        np.asarray(inputs["bc1"], np.float32),
        np.asarray(inputs["bc2"], np.float32),
        np.asarray(inputs["bo1"], np.float32),
        np.asarray(inputs["bo2"], np.float32),
        np.asarray(inputs["bih"], np.float32))
    from concourse.bass_utils import run_bass_kernel_spmd
    in_maps = [
        {"x": x[i * BL:(i + 1) * BL], "cond": cond[i * BL:(i + 1) * BL],
         "bands": bands}
        for i in range(NCORES)
    ]
    res = run_bass_kernel_spmd(nc, in_maps, list(range(NCORES)))
    out = np.concatenate(
        [res.results[i]["out"].reshape(BL) for i in range(NCORES)])
    return out.astype(np.float32)


if __name__ == "__main__":
    # smoke test with tiny random weights
    rng = np.random.default_rng(0)
    ins = {
        "x": rng.standard_normal((64, 16, 128, 128), np.float32),
        "cond": rng.standard_normal((64, 16, 128, 128), np.float32),
        "Wci": rng.standard_normal((3, 3, 1, 1), np.float32) * 0.1,
        "bci": np.zeros(1, np.float32),
        "Wc1": rng.standard_normal((3, 3, 16, 2), np.float32) * 0.1,
        "bc1": np.zeros(2, np.float32),
        "Wc2": rng.standard_normal((3, 3, 2, 2), np.float32) * 0.1,
        "bc2": np.zeros(2, np.float32),
        "Wo1": rng.standard_normal((3, 3, 4, 2), np.float32) * 0.1,
        "bo1": np.zeros(2, np.float32),
        "Wo2": rng.standard_normal((3, 3, 2, 2), np.float32) * 0.1,
        "bo2": np.zeros(2, np.float32),
        "Wih": rng.standard_normal((3, 3, 1, 8), np.float32) * 0.1,
        "bih": np.zeros(8, np.float32),
        "Whh": rng.standard_normal((3, 3, 2, 8), np.float32) * 0.1,
    }
    print(kernel(**ins)[:8])

